# revision 1
# baseline (speedup 1.0000x reference)
"""Trainium2 Bass kernel for nn_DSnetwork (GNN message passing).

Computation (see reference):
    3x layers: h = elu(h @ W + b + (segmean(h) @ Ws + bs)[batch_idx])
    out = relu(segmean(h) @ Wf1 + bf1) @ Wf2 + bf2

Strategy: batch_idx is sorted, so graphs are contiguous node ranges. Graphs are
split into 8 contiguous per-core ranges (node-balanced), and within a core into
chunks of <= C nodes covering whole graphs. Each chunk's full 3-layer network +
head is computed entirely in SBUF. Segment mean and the gather-broadcast are
matmuls against small host-built 0/1 indicator matrices (A: [node, graph] with
recip scaling, AT: [graph, node]). We carry v = 1 + elu(x) (always produced as
min(exp(x),1) + relu(x)) and fold the -1 into the next layer's bias via column
sums of W/Ws/Wf1.

v3 key points:
  - All feature-major -> node-major activation transposes go through the DMA
    xbar engine (dma_start_transpose): the xbar maps logical row n to
    (partition n%128, block n//128) - verified on device - matching the
    A-matrix host packing. This removes ~4 ldweights+matmul rounds per chunk
    from the PE and the psum->sbuf transpose copies from the DVE.
  - DMA issue cost (~650ns serialized HWDGE+SEQ per instruction) dominates if
    every chunk issues its own loads, so chunks are processed in groups of
    Q=4: one DMA per group for h/A/AT loads and for the SBUF->SBUF group
    transpose, one DRAM->SBUF transposing load per group for the layer-1
    feature-major activations, and a single final store of all head outputs.
  - The per-layer loop is software-pipelined: group g+1's loads/transposes are
    in flight while group g computes; within a group, chunk k+1's pooling
    matmuls issue before chunk k's x2/x1/gather, so the PE never waits on the
    pooledT/x2sb ACT/DVE copy round trips.
  - ELU work is spread over three queues: exp on ACT, relu on DVE, and the
    min/add combine on the otherwise-idle gpsimd queue.
"""

import os
import sys

for _p in ("/opt/trn_rl_repo", "/root/.axon_site/_ro/trn_rl_repo"):
    if os.path.isdir(_p) and _p not in sys.path:
        sys.path.insert(0, _p)

from contextlib import ExitStack
from dataclasses import dataclass

import numpy as np

import concourse.bass as bass
import concourse.mybir as mybir
import concourse.tile as tile
from concourse import bacc, bass_utils

F16 = mybir.dt.float16
F32 = mybir.dt.float32
AF = mybir.ActivationFunctionType
OP = mybir.AluOpType


@dataclass(frozen=True)
class Cfg:
    N: int = 500000
    D: int = 128
    G: int = 10000
    T: int = 10
    n_cores: int = 8
    C: int = 1024  # nodes per chunk (multiple of 128)
    GCH: int = 64  # max graphs per chunk
    Q: int = 4  # chunks per DMA group

    @property
    def BLK(self):
        return self.C // 128


CFG = Cfg()

RELU_SPLIT = 768  # cols of the relu pass on ACT (rest on DVE)


# --------------------------------------------------------------------------
# Host-side preparation
# --------------------------------------------------------------------------

def _prepare(cfg, h, batch_idx, W, b, Ws, bs, Wf1, bf1, Wf2, bf2):
    """Pack graphs into per-core chunk arrays. Returns in_maps + assembly info."""
    N, D, G, T, C, GCH = cfg.N, cfg.D, cfg.G, cfg.T, cfg.C, cfg.GCH
    BLK, Q = cfg.BLK, cfg.Q
    bi = np.asarray(batch_idx).astype(np.int64)
    counts = np.bincount(bi, minlength=G)
    starts = np.zeros(G + 1, np.int64)
    np.cumsum(counts, out=starts[1:])
    assert counts.max() <= C, "single graph larger than a chunk"

    # split graphs into n_cores contiguous ranges with ~equal node counts
    targets = (np.arange(1, cfg.n_cores) * (N / cfg.n_cores)).astype(np.int64)
    cuts = np.searchsorted(starts[1 : G + 1], targets)
    core_g = np.concatenate([[0], cuts, [G]])

    # chunk packing per core
    core_chunks = []
    for c in range(cfg.n_cores):
        g0, g1 = int(core_g[c]), int(core_g[c + 1])
        chunks = []
        g = g0
        while g < g1:
            ge = g
            nodes = 0
            while ge < g1 and ge - g < GCH and nodes + counts[ge] <= C:
                nodes += int(counts[ge])
                ge += 1
            assert ge > g
            chunks.append((g, ge))
            g = ge
        core_chunks.append(chunks)
    nchunk = max(len(ch) for ch in core_chunks)
    nchunk = (nchunk + Q - 1) // Q * Q  # pad to a whole number of groups

    # weights prep (shared across cores)
    W = [np.asarray(w, np.float32) for w in W]
    Ws = [np.asarray(w, np.float32) for w in Ws]
    b = [np.asarray(x, np.float32) for x in b]
    bs = [np.asarray(x, np.float32) for x in bs]
    Wf1 = np.asarray(Wf1, np.float32)
    bf1 = np.asarray(bf1, np.float32)
    Wf2 = np.asarray(Wf2, np.float32)
    bf2 = np.asarray(bf2, np.float32)

    brow = np.zeros((3, D), np.float32)
    for l in range(3):
        brow[l] = b[l] + bs[l]
        if l >= 1:  # inputs are v = h + 1 -> subtract column sums
            brow[l] -= W[l].sum(axis=0) + Ws[l].sum(axis=0)
    bf1_eff = bf1 - Wf1.sum(axis=0)  # pooled input is v = h + 1

    W_h = np.stack([w.astype(np.float16) for w in W])  # [3,128,128]
    Ws_h = np.stack([w.astype(np.float16) for w in Ws])
    bcol = brow.T.copy()  # [128, 3] f32, per-feature bias columns
    bf1_col = bf1_eff.reshape(2, D).T.copy()  # [128, 2]
    Wf2_r = Wf2.reshape(2, D, T).copy()  # [2, 128, 10]
    bf2_col = bf2.reshape(T, 1).copy()

    h = np.ascontiguousarray(np.asarray(h, np.float32)).astype(np.float16)
    in_maps = []
    asm = []  # per core: (positions into [nchunk*GCH], graph ids)
    for c in range(cfg.n_cores):
        chunks = core_chunks[c]
        h_pad = np.zeros((nchunk * C, D), np.float16)
        A = np.zeros((nchunk, 128, BLK, GCH), np.float16)
        AT = np.zeros((nchunk, GCH, C), np.float16)
        recip = np.zeros((nchunk, GCH), np.float32)
        pos_list = []
        gid_list = []
        for k, (gs, ge) in enumerate(chunks):
            n0, n1 = int(starts[gs]), int(starts[ge])
            nn = n1 - n0
            h_pad[k * C : k * C + nn] = h[n0:n1]
            lidx = (bi[n0:n1] - gs).astype(np.int64)  # local graph idx per node
            narng = np.arange(nn)
            ng = ge - gs
            recip[k, :ng] = 1.0 / np.maximum(counts[gs:ge], 1)
            A[k, narng % 128, narng // 128, lidx] = recip[k, lidx].astype(np.float16)
            AT[k, lidx, narng] = 1.0
            pos_list.append(k * GCH + np.arange(ng))
            gid_list.append(np.arange(gs, ge))
        # node-major pre-blocked copy: hb[k, p, b, :] = h_pad[k*C + b*128 + p]
        # (one contiguous 2KB descriptor per partition instead of 8x 256B)
        h_blk = np.ascontiguousarray(
            h_pad.reshape(nchunk, BLK, 128, D).transpose(0, 2, 1, 3)
        )
        # group-major reshapes so one DMA covers Q chunks
        A_g = np.ascontiguousarray(
            A.reshape(nchunk // Q, Q, 128, BLK, GCH).transpose(0, 2, 1, 3, 4)
        )  # [ngrp, 128, Q, BLK, GCH]
        AT_g = np.ascontiguousarray(
            AT.reshape(nchunk // Q, Q, GCH, C).transpose(0, 2, 1, 3)
        )  # [ngrp, GCH, Q, C]
        hb_g = np.ascontiguousarray(
            h_blk.reshape(nchunk // Q, Q, 128, BLK, D).transpose(0, 2, 1, 3, 4)
        )  # [ngrp, 128, Q, BLK, D]
        in_maps.append(
            {
                "h": h_pad,
                "hb": hb_g,
                "A": A_g,
                "AT": AT_g,
                "W": W_h,
                "Wsm": Ws_h,
                "bcol": bcol,
                "wf1": Wf1.astype(np.float16),
                "bf1c": bf1_col,
                "wf2": Wf2_r,
                "bf2c": bf2_col,
            }
        )
        asm.append(
            (
                np.concatenate(pos_list) if pos_list else np.zeros(0, np.int64),
                np.concatenate(gid_list) if gid_list else np.zeros(0, np.int64),
            )
        )

    # rows for empty graphs (reference: pooled = 0)
    empty_row = (
        np.maximum(bf1, 0.0) @ Wf2 + bf2 if (counts == 0).any() else None
    )
    return {
        "nchunk": nchunk,
        "in_maps": in_maps,
        "asm": asm,
        "counts": counts,
        "empty_row": empty_row,
    }


# --------------------------------------------------------------------------
# Device program
# --------------------------------------------------------------------------

def _build(cfg, nchunk, reps=1):
    """Build the Bass program. reps>1 wraps the body in a repeat loop (timing)."""
    D, T, C, GCH, BLK, Q = cfg.D, cfg.T, cfg.C, cfg.GCH, cfg.BLK, cfg.Q
    ngrp = nchunk // Q
    nc = bacc.Bacc("TRN2", target_bir_lowering=False, debug=False)

    h_d = nc.dram_tensor("h", [nchunk * C, D], F16, kind="ExternalInput").ap()
    hb_d = nc.dram_tensor("hb", [ngrp, 128, Q, BLK, D], F16, kind="ExternalInput").ap()
    A_d = nc.dram_tensor("A", [ngrp, 128, Q, BLK, GCH], F16, kind="ExternalInput").ap()
    AT_d = nc.dram_tensor("AT", [ngrp, GCH, Q, C], F16, kind="ExternalInput").ap()
    W_d = nc.dram_tensor("W", [3, D, D], F16, kind="ExternalInput").ap()
    Ws_d = nc.dram_tensor("Wsm", [3, D, D], F16, kind="ExternalInput").ap()
    bcol_d = nc.dram_tensor("bcol", [D, 3], F32, kind="ExternalInput").ap()
    wf1_d = nc.dram_tensor("wf1", [D, 2 * D], F16, kind="ExternalInput").ap()
    bf1_d = nc.dram_tensor("bf1c", [D, 2], F32, kind="ExternalInput").ap()
    wf2_d = nc.dram_tensor("wf2", [2, D, T], F32, kind="ExternalInput").ap()
    bf2_d = nc.dram_tensor("bf2c", [T, 1], F32, kind="ExternalInput").ap()
    out_d = nc.dram_tensor("out", [T, nchunk * GCH], F32, kind="ExternalOutput").ap()

    with tile.TileContext(nc) as tc, ExitStack() as ctx:
        const = ctx.enter_context(tc.tile_pool(name="const", bufs=1))
        io = ctx.enter_context(tc.tile_pool(name="io", bufs=2))
        hTp = ctx.enter_context(tc.tile_pool(name="hTp", bufs=2))
        wk = ctx.enter_context(tc.tile_pool(name="wk", bufs=3))
        sm = ctx.enter_context(tc.tile_pool(name="sm", bufs=4))
        # PSUM: xb 2 banks x 3 bufs + packed small bank x 2 bufs = 8 banks
        ps_big = ctx.enter_context(tc.tile_pool(name="ps_big", bufs=3, space="PSUM"))
        ps_blk = ctx.enter_context(tc.tile_pool(name="ps_blk", bufs=2, space="PSUM"))

        W_sb = const.tile([D, 3, D], F16, name="W_sb")
        nc.sync.dma_start(W_sb[:], W_d.rearrange("l k m -> k l m"))
        Ws_sb = const.tile([D, 3, D], F16, name="Ws_sb")
        nc.sync.dma_start(Ws_sb[:], Ws_d.rearrange("l k m -> k l m"))
        wf1_sb = const.tile([D, 2 * D], F16, name="wf1_sb")
        nc.sync.dma_start(wf1_sb[:], wf1_d)
        bf1_sb = const.tile([D, 2], F32, name="bf1_sb")
        nc.sync.dma_start(bf1_sb[:], bf1_d)
        wf2_sb = const.tile([D, 2, T], F32, name="wf2_sb")
        nc.sync.dma_start(wf2_sb[:], wf2_d.rearrange("x k m -> k x m"))
        bf2_sb = const.tile([T, 1], F32, name="bf2_sb")
        nc.sync.dma_start(bf2_sb[:], bf2_d)
        bcol_sb = const.tile([D, 3], F32, name="bcol_sb")
        nc.sync.dma_start(bcol_sb[:], bcol_d)
        out_stage = const.tile([T, nchunk * GCH], F32, name="out_stage")

        vres = ctx.enter_context(tc.tile_pool(name="vres", bufs=1))

        def pool_mm(l, hT, A_sb, q):
            """pooledT[f, g] = sum_n hT[n, f] A[n, g] (feature-major)."""
            blk = ps_blk.tile([128, 512], F32, tag="blk")
            pool_ps = blk[:, 0:GCH]
            for bb in range(BLK):
                nc.tensor.matmul(
                    pool_ps, hT[:, q, bb, :], A_sb[:, q, bb, :],
                    start=(bb == 0), stop=(bb == BLK - 1),
                )
            pooledT = sm.tile([128, GCH], F16, tag="pooledT")
            nc.scalar.copy(pooledT[:], pool_ps)
            return pooledT, blk

        def x2_mm(l, pooledT, blk):
            x2_ps = blk[:GCH, GCH : GCH + D]
            nc.tensor.matmul(x2_ps, pooledT[:], Ws_sb[:, l, :], start=True, stop=True)
            x2sb = sm.tile([GCH, D], F16, tag="x2sb")
            nc.vector.tensor_copy(x2sb[:], x2_ps)
            return x2sb

        def xb_elu(l, v_k, x2sb, AT_sb, q):
            xb_ps = ps_big.tile([128, C], F32, tag="xb")
            for ss in range(0, C, 512):
                nc.tensor.matmul(
                    xb_ps[:, ss : ss + 512], W_sb[:, l, :], v_k[:, ss : ss + 512],
                    start=True, stop=False,
                )
            for ss in range(0, C, 512):
                nc.tensor.matmul(
                    xb_ps[:, ss : ss + 512], x2sb[:], AT_sb[:, q, ss : ss + 512],
                    start=False, stop=True,
                )
            e_sb = wk.tile([128, C], F16, tag="e")
            nc.scalar.activation(e_sb[:], xb_ps[:], AF.Exp, bias=bcol_sb[:, l : l + 1])
            r_sb = wk.tile([128, C], F16, tag="r")
            if RELU_SPLIT <= 0:
                nc.vector.tensor_scalar(
                    r_sb[:], xb_ps[:], bcol_sb[:, l : l + 1], 0.0, OP.add, OP.max
                )
            elif RELU_SPLIT >= C:
                nc.scalar.activation(r_sb[:], xb_ps[:], AF.Relu, bias=bcol_sb[:, l : l + 1])
            else:
                nc.scalar.activation(
                    r_sb[:, :RELU_SPLIT], xb_ps[:, :RELU_SPLIT], AF.Relu,
                    bias=bcol_sb[:, l : l + 1],
                )
                nc.vector.tensor_scalar(
                    r_sb[:, RELU_SPLIT:], xb_ps[:, RELU_SPLIT:],
                    bcol_sb[:, l : l + 1], 0.0, OP.add, OP.max
                )
            nc.vector.scalar_tensor_tensor(v_k[:], e_sb[:], 1.0, r_sb[:], OP.min, OP.add)

        def head_rest(k, pooledT, blk):
            r1_sbs = []
            r1t = ps_big.tile([128, C], F32, tag="xb")
            r1_ps = r1t[:, 0 : 2 * GCH]
            for hh in range(2):
                nc.tensor.matmul(
                    r1_ps[:, hh * GCH : (hh + 1) * GCH],
                    wf1_sb[:, hh * 128 : (hh + 1) * 128], pooledT[:],
                    start=True, stop=True,
                )
                r1_sb = sm.tile([128, GCH], F32, tag=f"r1s_{hh}")
                nc.scalar.activation(
                    r1_sb[:], r1_ps[:, hh * GCH : (hh + 1) * GCH], AF.Relu,
                    bias=bf1_sb[:, hh : hh + 1],
                )
                r1_sbs.append(r1_sb)
            out_ps = blk[:T, GCH + D : GCH + D + GCH]
            for hh in range(2):
                nc.tensor.matmul(
                    out_ps, wf2_sb[:, hh, :], r1_sbs[hh][:],
                    start=(hh == 0), stop=(hh == 1),
                )
            nc.scalar.activation(
                out_stage[:, k * GCH : (k + 1) * GCH], out_ps, AF.Identity,
                bias=bf2_sb[:],
            )

        def body_lm():
            vgs = [
                vres.tile([128, Q * C], F16, tag=f"v{j}", name=f"v{j}")
                for j in range(ngrp)
            ]

            def issue_loads(l, j, need_at=True):
                hT = hTp.tile([128, Q, BLK, 128], F16, tag="hT")
                if l == 0:
                    nc.sync.dma_start(hT[:], hb_d[j])
                    nc.sync.dma_start_transpose(
                        vgs[j][:], h_d[j * Q * C : (j + 1) * Q * C, :]
                    )
                else:
                    nc.sync.dma_start_transpose(hT[:], vgs[j][:])
                A_sb = io.tile([128, Q, BLK, GCH], F16, tag="A")
                nc.sync.dma_start(A_sb[:], A_d[j])
                AT_sb = None
                if need_at:
                    AT_sb = io.tile([GCH, Q, C], F16, tag="AT")
                    nc.sync.dma_start(AT_sb[:], AT_d[j])
                return (hT, A_sb, AT_sb)

            def sweep(l, need_at=True):
                loaded = [issue_loads(l, j, need_at) for j in range(min(2, ngrp))]
                pend = None
                for j in range(ngrp):
                    hT, A_sb, AT_sb = loaded[j]
                    if j + 2 < ngrp:
                        loaded.append(issue_loads(l, j + 2, need_at))
                    for q in range(Q):
                        k = j * Q + q
                        pooledT, blk = pool_mm(l, hT, A_sb, q)
                        if pend is not None:
                            if l < 3:
                                pk, p_pooledT, p_blk, p_AT, p_q, p_vg = pend
                                x2sb = x2_mm(l, p_pooledT, p_blk)
                                xb_elu(
                                    l,
                                    p_vg[:, p_q * C : (p_q + 1) * C],
                                    x2sb, p_AT, p_q,
                                )
                            else:
                                pk, p_pooledT, p_blk, _, _, _ = pend
                                head_rest(pk, p_pooledT, p_blk)
                        pend = (k, pooledT, blk, AT_sb, q, vgs[j])
                if pend is not None:
                    if l < 3:
                        pk, p_pooledT, p_blk, p_AT, p_q, p_vg = pend
                        x2sb = x2_mm(l, p_pooledT, p_blk)
                        xb_elu(l, p_vg[:, p_q * C : (p_q + 1) * C], x2sb, p_AT, p_q)
                    else:
                        pk, p_pooledT, p_blk, _, _, _ = pend
                        head_rest(pk, p_pooledT, p_blk)

            for l in range(3):
                sweep(l)
            sweep(3, need_at=False)
            nc.sync.dma_start(out_d, out_stage[:])

        if reps > 1:
            with tc.For_i(0, reps, 1):
                body_lm()
        else:
            body_lm()

    nc._tc_dbg = tc.ordered_instructions_by_block
    nc.compile()
    return nc


# --------------------------------------------------------------------------
# Entry point
# --------------------------------------------------------------------------

_CACHE = {}


def _run(cfg, inputs, reps=1):
    prep = _prepare(
        cfg,
        inputs["h_subgraph"],
        inputs["batch_idx"],
        [inputs["W1"], inputs["W2"], inputs["W3"]],
        [inputs["b1"], inputs["b2"], inputs["b3"]],
        [inputs["Ws1"], inputs["Ws2"], inputs["Ws3"]],
        [inputs["bs1"], inputs["bs2"], inputs["bs3"]],
        inputs["Wf1"],
        inputs["bf1"],
        inputs["Wf2"],
        inputs["bf2"],
    )
    key = (cfg, prep["nchunk"], reps)
    if key not in _CACHE:
        _CACHE[key] = _build(cfg, prep["nchunk"], reps=reps)
    nc = _CACHE[key]
    res = bass_utils.run_bass_kernel_spmd(
        nc, prep["in_maps"], core_ids=list(range(cfg.n_cores))
    )
    out = np.zeros((cfg.G, cfg.T), np.float32)
    for c in range(cfg.n_cores):
        oc = res.results[c]["out"]  # [T, nchunk*GCH]
        pos, gid = prep["asm"][c]
        if len(pos):
            out[gid, :] = oc[:, pos].T
    if prep["empty_row"] is not None:
        out[prep["counts"] == 0, :] = prep["empty_row"]
    return out


def kernel(**inputs):
    return _run(CFG, inputs, reps=1).astype(np.float32)

